# revision 12
# baseline (speedup 1.0000x reference)
"""BallLoss Trainium2 kernel (8-core data-parallel SPMD), v5.

loss = sum_{i,j} relu(d_i - d_ij),  d_ij = ||e_i - c_j||, d_i = d_{i,label_i}

Per-core (rows sharded along N across 8 cores, centers replicated):

  - PE:   p[i,j] = c2_j - 2*e_i.c_j via an augmented bf16 matmul, K=65:
          lhsT = [-2*e_i; 1] (host supplies the -2 scale on the e side),
          rhs  = [c^T; c2] (c^T DMA'd straight from the host in bf16;
          c2 computed on-device into psum partition 64 via ones-matmuls
          and copied across to chat row 64 on the same partition).
  - ACT:  dist = sqrt(p + e2_i) (bias = e2 per partition), PSUM -> SBUF
          bf16, one op per [128, 2048] row-tile (~1.86us, the ACT rate
          is 1x from fp32 PSUM).
  - DVE:  z' = (dist min d_i) - d_i = -relu(d_i - dist), one fused
          tensor_scalar at the 4x bf16 rate (~0.73us). The loss is the
          plain GRAND sum of z', so most tiles (H) then do a 2x
          tensor_tensor add into one of several persistent bf16
          accumulators (~1.23us); accumulating the RELU (mean ~-0.56)
          instead of min (~11) keeps bf16 rounding bias ~1e-4 instead
          of a catastrophic 0.5% on the cancelling min-sum. A few
          tiles (A) instead go ACT Identity+accum -> macc, and a few
          H adds run on the otherwise idle GpSimd engine, balancing
          ACT ~ DVE ~ 140us.
  - d_i:  from per-tile indirect-DMA gathers of c[label] (fp32):
          d2_i = sum_d (e_id - c_{lab_i,d})^2 in bf16, d_i = sqrt(d2_i).
  - final: loss = -(sum macc + sum of all accumulators).

Scheduling: input DMAs up front (cTb first: it gates the chat build);
the gather + d2 precompute chain for each 8-tile group is emitted a
full group AHEAD of that group's main tiles so the in-order ACT/DVE
streams never wait on the (mismodeled, slow) indirect gathers; explicit
ordering deps still pin each chain behind earlier main DVE ops so the
scheduler cannot hoist it into the startup. A-tiles are placed at the
very end so the final accumulator merges/reduce overlap their ACT work.

Host: shards inputs, provides layouts/casts only (e^T scaled by -2 in
bf16, ones row, bf16 e, labels as int32), sums the 8 per-core scalars.
"""

from contextlib import ExitStack

import ml_dtypes
import numpy as np

import concourse.bass as bass
import concourse.tile as tile
from concourse import bacc, mybir
from concourse.bass_utils import run_bass_kernel_spmd

F32 = mybir.dt.float32
BF16 = mybir.dt.bfloat16
I32 = mybir.dt.int32
AF = mybir.ActivationFunctionType
OP = mybir.AluOpType
AX = mybir.AxisListType

N, C, D = 65536, 2048, 64
NCORES = 8
NS = N // NCORES  # 8192 rows per core
P = 128           # partitions
T = NS // P       # 64 row-tiles per core
FD = 512          # fp32 psum bank free dim
NB = C // FD      # 4 matmuls per row-tile
G = 8             # row-tiles per precompute group
NG = T // G       # 8 groups

MM_DT = BF16
KA = D + 1        # 64 e dims + c2 ones row

# ACT-heavy tiles (ACT id+accum). The last ones overlap the tail merges.
A_TILES = frozenset({4, 12, 20, 28, 36, 44, 52, 61, 63})
# H tiles whose accumulator add runs on GpSimd (own accumulator)
POOL_TILES = frozenset()
NSPLIT = 24       # DVE H-tiles before switching accumulator pair


def _body(tc, out, eT, enat, labT, cTb, cnat):
    nc = tc.nc
    with ExitStack() as ctx:
        const = ctx.enter_context(tc.tile_pool(name="const", bufs=1))

        eTa = const.tile([KA, NS], MM_DT)    # [65, 8192]
        chat = const.tile([KA, C], MM_DT)    # [65, 2048]: c^T rows + c2
        csqb = const.tile([D, C], BF16)
        ensb = const.tile([P, T * D], BF16)  # e natural, tile-major
        clab = const.tile([P, T * D], F32)   # gathered centers per row
        cdif = const.tile([P, T * D], BF16)  # e - c[label], bf16
        scrb = const.tile([P, T * D], BF16)  # scratch squares
        labsb = const.tile([P, T], I32)
        ones = const.tile([P, 1], BF16)
        onesf = const.tile([P, 1], F32)
        zaccs = [const.tile([P, C], BF16, name=f"zacc{i}") for i in range(4)]
        e2 = const.tile([P, T], F32)
        d2 = const.tile([P, T], F32)
        dall = const.tile([P, T], F32)
        macc = const.tile([P, T], F32)
        rowtot = const.tile([P, 1], F32)
        zrow = const.tile([P, 1], F32)
        outsb = const.tile([1, 1], F32)

        # DMA queue order tuned for the startup critical paths:
        # labsb gates the (slow) indirect gathers; ensb g0 gates the d2
        # chain; cTb gates the chat build; eTa g0 gates the first mains.
        def dma_ensb(g):
            fs, fe = g * G * D, (g + 1) * G * D
            cs, ce = g * G * P, (g + 1) * G * P
            nc.sync.dma_start(
                ensb[:, fs:fe].rearrange("p (t d) -> p t d", d=D),
                enat[cs:ce, :].rearrange("(t p) d -> p t d", p=P),
            )

        def dma_eTa(g):
            cs, ce = g * G * P, (g + 1) * G * P
            nc.sync.dma_start(eTa[:, cs:ce], eT[:, cs:ce])

        nc.sync.dma_start(labsb[:], labT)
        dma_ensb(0)
        nc.sync.dma_start(chat[0:D, :], cTb)
        dma_eTa(0)
        nc.vector.memset(ones[:], 1.0)
        nc.vector.memset(onesf[:], 1.0)
        nc.vector.memset(macc[:], 0.0)
        for g in range(1, NG):
            dma_ensb(g)
            dma_eTa(g)

        mm_ctx = tc.tile_pool(name="mm", bufs=2, space="PSUM")
        mm_pool = mm_ctx.__enter__()

        # c2 row: csq = (c^T)^2, ones-matmul column sums into psum
        # partition 64, one copy across to chat row 64 (same partition).
        c2ps_full = mm_pool.tile([P, C], F32, name="ps", tag="ps")
        nc.vector.tensor_mul(csqb[:], chat[0:D, :], chat[0:D, :])
        for k in range(NB):
            sl = slice(k * FD, (k + 1) * FD)
            nc.tensor.matmul(
                c2ps_full[64:65, sl], lhsT=ones[0:D, :], rhs=csqb[:, sl],
                start=True, stop=True,
            )
        c2_i = nc.vector.tensor_copy(chat[D:KA, :], c2ps_full[64:65, :])

        dist_pool = ctx.enter_context(tc.tile_pool(name="dist", bufs=6))
        from concourse.tile import add_dep_helper

        z_insts = []
        state = {"nh": 0, "np": 0}
        bounds = [(4 * k, 4 * (k + 1)) for k in range(T // 4)]

        def pre(bi):
            ts, te = bounds[bi]
            fs, fe = ts * D, te * D
            for t in range(ts, te):
                nc.gpsimd.indirect_dma_start(
                    out=clab[:, t * D:(t + 1) * D],
                    out_offset=None,
                    in_=cnat,
                    in_offset=bass.IndirectOffsetOnAxis(
                        ap=labsb[:, t:t + 1], axis=0),
                )
            nc.vector.tensor_mul(scrb[:, fs:fe], ensb[:, fs:fe],
                                 ensb[:, fs:fe])
            nc.vector.tensor_reduce(
                e2[:, ts:te],
                scrb[:, fs:fe].rearrange("p (t d) -> p t d", d=D),
                axis=AX.X, op=OP.add,
            )
            nc.gpsimd.tensor_sub(
                cdif[:, fs:fe], ensb[:, fs:fe], clab[:, fs:fe]
            )
            sub_i = nc.vector.tensor_mul(
                scrb[:, fs:fe], cdif[:, fs:fe], cdif[:, fs:fe]
            )
            if len(z_insts) >= 3:
                add_dep_helper(sub_i.ins, z_insts[-3].ins, sync=False,
                               reason="hold d2 chain behind prior tiles")
            else:
                add_dep_helper(sub_i.ins, c2_i.ins, sync=False,
                               reason="hold early d2 chain behind chat")
            nc.vector.tensor_reduce(
                d2[:, ts:te],
                scrb[:, fs:fe].rearrange("p (t d) -> p t d", d=D),
                axis=AX.X, op=OP.add,
            )
            nc.scalar.activation(dall[:, ts:te], d2[:, ts:te], AF.Sqrt)

        def tiles(bi):
            ts, te = bounds[bi]
            for t in range(ts, te):
                ps = mm_pool.tile([P, C], F32, name="ps")
                lhsT = eTa[:, t * P:(t + 1) * P]
                for k in range(NB):
                    nc.tensor.matmul(
                        ps[:, k * FD:(k + 1) * FD],
                        lhsT=lhsT,
                        rhs=chat[:, k * FD:(k + 1) * FD],
                        start=True, stop=True,
                    )
                xz = dist_pool.tile([P, C], BF16, name="dist")
                nc.scalar.activation(
                    xz[:], ps[:], AF.Sqrt,
                    bias=e2[:, t:t + 1], scale=1.0,
                )
                zi = nc.vector.tensor_scalar(
                    out=xz[:], in0=xz[:],
                    scalar1=dall[:, t:t + 1], scalar2=dall[:, t:t + 1],
                    op0=OP.min, op1=OP.subtract,
                )
                z_insts.append(zi)
                if t in A_TILES:
                    nc.scalar.activation(
                        xz[:], xz[:], AF.Identity,
                        accum_out=macc[:, t:t + 1],
                    )
                elif t in POOL_TILES:
                    if state["np"] == 0:
                        nc.gpsimd.tensor_copy(zaccs[4][:], xz[:])
                    else:
                        nc.gpsimd.tensor_add(zaccs[4][:], zaccs[4][:],
                                             xz[:])
                    state["np"] += 1
                else:
                    nh = state["nh"]
                    if nh == NSPLIT:
                        nc.vector.tensor_add(zaccs[0][:], zaccs[0][:],
                                             zaccs[1][:])
                    za = zaccs[nh % 2] if nh < NSPLIT else zaccs[2 + nh % 2]
                    first = nh < 2 or (NSPLIT <= nh < NSPLIT + 2)
                    if first:
                        nc.vector.tensor_copy(za[:], xz[:])
                    else:
                        nc.vector.tensor_add(za[:], za[:], xz[:])
                    state["nh"] += 1

        for bi in range(len(bounds)):
            pre(bi)
            tiles(bi)

        mm_ctx.__exit__(None, None, None)

        # merge accumulators, reduce, negate
        nc.vector.tensor_add(zaccs[2][:], zaccs[2][:], zaccs[3][:])
        nc.vector.tensor_add(zaccs[0][:], zaccs[0][:], zaccs[2][:])
        nc.vector.tensor_reduce(rowtot[:], macc[:], axis=AX.X, op=OP.add)
        nc.vector.tensor_reduce(zrow[:], zaccs[0][:], axis=AX.X, op=OP.add)
        nc.vector.tensor_add(rowtot[:], rowtot[:], zrow[:])
        nc.vector.tensor_scalar_mul(rowtot[:], rowtot[:], -1.0)
        with tc.tile_pool(name="fin", bufs=1, space="PSUM") as finp:
            fin = finp.tile([1, 1], F32)
            nc.tensor.matmul(fin[:], lhsT=rowtot[:], rhs=onesf[:],
                             start=True, stop=True)
            nc.scalar.copy(outsb[:], fin[:])
        nc.sync.dma_start(out, outsb[:])


_NC_CACHE = {}


def build_nc():
    if "nc" in _NC_CACHE:
        return _NC_CACHE["nc"]
    nc = bacc.Bacc(
        "TRN2", target_bir_lowering=False, debug=False, enable_asserts=False
    )
    eT = nc.dram_tensor("eT", [KA, NS], MM_DT, kind="ExternalInput").ap()
    enat = nc.dram_tensor("enat", [NS, D], BF16, kind="ExternalInput").ap()
    labT = nc.dram_tensor("labT", [P, T], I32, kind="ExternalInput").ap()
    cTb = nc.dram_tensor("cTb", [D, C], BF16, kind="ExternalInput").ap()
    cnat = nc.dram_tensor("cnat", [C, D], F32, kind="ExternalInput").ap()
    out = nc.dram_tensor("out", [1, 1], F32, kind="ExternalOutput").ap()
    with nc.allow_low_precision(reason="bf16 distance pipeline"):
        with tile.TileContext(nc) as tc:
            _body(tc, out, eT, enat, labT, cTb, cnat)
    nc.compile()
    _NC_CACHE["nc"] = nc
    return nc


def make_in_maps(embeddings, centers, labels):
    e = np.ascontiguousarray(np.asarray(embeddings, dtype=np.float32))
    c = np.ascontiguousarray(np.asarray(centers, dtype=np.float32))
    lab = np.asarray(labels).astype(np.int32)
    assert e.shape == (N, D) and c.shape == (C, D) and lab.shape == (N,)
    cTb = np.ascontiguousarray(c.T).astype(ml_dtypes.bfloat16)
    in_maps = []
    for core in range(NCORES):
        es = e[core * NS:(core + 1) * NS]
        ls = lab[core * NS:(core + 1) * NS]
        eT65 = np.ones((KA, NS), np.float32)
        eT65[0:D] = -2.0 * es.T
        eT65 = eT65.astype(ml_dtypes.bfloat16)
        in_maps.append({
            "eT": eT65,
            "enat": np.ascontiguousarray(es.astype(ml_dtypes.bfloat16)),
            "labT": np.ascontiguousarray(ls.reshape(T, P).T),
            "cTb": cTb,
            "cnat": c,
        })
    return in_maps


def run(embeddings, centers, labels, **kw):
    nc = build_nc()
    in_maps = make_in_maps(embeddings, centers, labels)
    res = run_bass_kernel_spmd(nc, in_maps, core_ids=list(range(NCORES)), **kw)
    total = float(sum(float(r["out"][0, 0]) for r in res.results))
    return np.float32(total), res


def kernel(embeddings, centers, labels):
    val, _ = run(embeddings, centers, labels)
    return val


# revision 13
# speedup vs baseline: 1.1986x; 1.1986x over previous
"""BallLoss Trainium2 kernel (8-core data-parallel SPMD), v5.

loss = sum_{i,j} relu(d_i - d_ij),  d_ij = ||e_i - c_j||, d_i = d_{i,label_i}

Per-core (rows sharded along N across 8 cores, centers replicated):

  - PE:   p[i,j] = c2_j - 2*e_i.c_j via an augmented bf16 matmul, K=65:
          lhsT = [-2*e_i; 1] (host supplies the -2 scale on the e side),
          rhs  = [c^T; c2] (c^T DMA'd straight from the host in bf16;
          c2 computed on-device into psum partition 64 via ones-matmuls
          and copied across to chat row 64 on the same partition).
  - ACT:  dist = sqrt(p + e2_i) (bias = e2 per partition), PSUM -> SBUF
          bf16, one op per [128, 2048] row-tile (~1.86us, the ACT rate
          is 1x from fp32 PSUM).
  - DVE:  z' = (dist min d_i) - d_i = -relu(d_i - dist), one fused
          tensor_scalar at the 4x bf16 rate (~0.73us). The loss is the
          plain GRAND sum of z', so most tiles (H) then do a 2x
          tensor_tensor add into one of several persistent bf16
          accumulators (~1.23us); accumulating the RELU (mean ~-0.56)
          instead of min (~11) keeps bf16 rounding bias ~1e-4 instead
          of a catastrophic 0.5% on the cancelling min-sum. A few
          tiles (A) instead go ACT Identity+accum -> macc, and a few
          H adds run on the otherwise idle GpSimd engine, balancing
          ACT ~ DVE ~ 140us.
  - d_i:  from per-tile indirect-DMA gathers of c[label] (fp32):
          d2_i = sum_d (e_id - c_{lab_i,d})^2 in bf16, d_i = sqrt(d2_i).
  - final: loss = -(sum macc + sum of all accumulators).

Scheduling: input DMAs up front (cTb first: it gates the chat build);
the gather + d2 precompute chain for each 8-tile group is emitted a
full group AHEAD of that group's main tiles so the in-order ACT/DVE
streams never wait on the (mismodeled, slow) indirect gathers; explicit
ordering deps still pin each chain behind earlier main DVE ops so the
scheduler cannot hoist it into the startup. A-tiles are placed at the
very end so the final accumulator merges/reduce overlap their ACT work.

Host: shards inputs, provides layouts/casts only (e^T scaled by -2 in
bf16, ones row, bf16 e, labels as int32), sums the 8 per-core scalars.
"""

from contextlib import ExitStack

import ml_dtypes
import numpy as np

import concourse.bass as bass
import concourse.tile as tile
from concourse import bacc, mybir
from concourse.bass_utils import run_bass_kernel_spmd

F32 = mybir.dt.float32
BF16 = mybir.dt.bfloat16
I32 = mybir.dt.int32
AF = mybir.ActivationFunctionType
OP = mybir.AluOpType
AX = mybir.AxisListType

N, C, D = 65536, 2048, 64
NCORES = 8
NS = N // NCORES  # 8192 rows per core
P = 128           # partitions
T = NS // P       # 64 row-tiles per core
FD = 512          # fp32 psum bank free dim
NB = C // FD      # 4 matmuls per row-tile
G = 8             # row-tiles per precompute group
NG = T // G       # 8 groups

MM_DT = BF16
KA = D + 1        # 64 e dims + c2 ones row

# ACT-heavy tiles (ACT id+accum). The last ones overlap the tail merges.
A_TILES = frozenset({4, 12, 20, 28, 36, 44, 52, 61, 63})
# H tiles whose accumulator add runs on GpSimd (own accumulator)
POOL_TILES = frozenset()
NSPLIT = 24       # DVE H-tiles before switching accumulator pair


def _body(tc, out, eT, enat, labT, cTb, cnat):
    nc = tc.nc
    with ExitStack() as ctx:
        const = ctx.enter_context(tc.tile_pool(name="const", bufs=1))

        eTa = const.tile([KA, NS], MM_DT)    # [65, 8192]
        chat = const.tile([KA, C], MM_DT)    # [65, 2048]: c^T rows + c2
        csqb = const.tile([D, C], BF16)
        ensb = const.tile([P, T * D], BF16)  # e natural, tile-major
        clab = const.tile([P, T * D], F32)   # gathered centers per row
        cdif = const.tile([P, T * D], BF16)  # e - c[label], bf16
        scrb = const.tile([P, T * D], BF16)  # scratch squares
        labsb = const.tile([P, T], I32)
        ones = const.tile([P, 1], BF16)
        onesf = const.tile([P, 1], F32)
        zaccs = [const.tile([P, C], BF16, name=f"zacc{i}") for i in range(4)]
        e2 = const.tile([P, T], F32)
        d2 = const.tile([P, T], F32)
        dall = const.tile([P, T], F32)
        macc = const.tile([P, T], F32)
        rowtot = const.tile([P, 1], F32)
        zrow = const.tile([P, 1], F32)
        outsb = const.tile([1, 1], F32)

        # DMA queue order tuned for the startup critical paths:
        # labsb gates the (slow) indirect gathers; ensb g0 gates the d2
        # chain; cTb gates the chat build; eTa g0 gates the first mains.
        def dma_ensb(g):
            fs, fe = g * G * D, (g + 1) * G * D
            nc.sync.dma_start(ensb[:, fs:fe], enat[:, fs:fe])

        def dma_eTa(g):
            cs, ce = g * G * P, (g + 1) * G * P
            nc.sync.dma_start(eTa[:, cs:ce], eT[:, cs:ce])

        nc.sync.dma_start(labsb[:], labT)
        nc.sync.dma_start(chat[0:D, :], cTb)
        dma_ensb(0)
        dma_eTa(0)
        nc.vector.memset(ones[:], 1.0)
        nc.vector.memset(onesf[:], 1.0)
        nc.vector.memset(macc[:], 0.0)
        for g in range(1, NG):
            dma_ensb(g)
            dma_eTa(g)

        mm_ctx = tc.tile_pool(name="mm", bufs=2, space="PSUM")
        mm_pool = mm_ctx.__enter__()

        # c2 row: csq = (c^T)^2, ones-matmul column sums into psum
        # partition 64, one copy across to chat row 64 (same partition).
        c2ps_full = mm_pool.tile([P, C], F32, name="ps", tag="ps")
        nc.vector.tensor_mul(csqb[:], chat[0:D, :], chat[0:D, :])
        for k in range(NB):
            sl = slice(k * FD, (k + 1) * FD)
            nc.tensor.matmul(
                c2ps_full[64:65, sl], lhsT=ones[0:D, :], rhs=csqb[:, sl],
                start=True, stop=True,
            )
        c2_i = nc.vector.tensor_copy(chat[D:KA, :], c2ps_full[64:65, :])

        dist_pool = ctx.enter_context(tc.tile_pool(name="dist", bufs=6))
        from concourse.tile import add_dep_helper

        z_insts = []
        state = {"nh": 0, "np": 0}
        bounds = [(4 * k, 4 * (k + 1)) for k in range(T // 4)]

        def pre(bi):
            ts, te = bounds[bi]
            fs, fe = ts * D, te * D
            for t in range(ts, te):
                nc.gpsimd.indirect_dma_start(
                    out=clab[:, t * D:(t + 1) * D],
                    out_offset=None,
                    in_=cnat,
                    in_offset=bass.IndirectOffsetOnAxis(
                        ap=labsb[:, t:t + 1], axis=0),
                )
            nc.vector.tensor_mul(scrb[:, fs:fe], ensb[:, fs:fe],
                                 ensb[:, fs:fe])
            nc.vector.tensor_reduce(
                e2[:, ts:te],
                scrb[:, fs:fe].rearrange("p (t d) -> p t d", d=D),
                axis=AX.X, op=OP.add,
            )
            nc.gpsimd.tensor_sub(
                cdif[:, fs:fe], ensb[:, fs:fe], clab[:, fs:fe]
            )
            sub_i = nc.vector.tensor_mul(
                scrb[:, fs:fe], cdif[:, fs:fe], cdif[:, fs:fe]
            )
            if len(z_insts) >= 3:
                add_dep_helper(sub_i.ins, z_insts[-3].ins, sync=False,
                               reason="hold d2 chain behind prior tiles")
            else:
                add_dep_helper(sub_i.ins, c2_i.ins, sync=False,
                               reason="hold early d2 chain behind chat")
            nc.vector.tensor_reduce(
                d2[:, ts:te],
                scrb[:, fs:fe].rearrange("p (t d) -> p t d", d=D),
                axis=AX.X, op=OP.add,
            )
            nc.scalar.activation(dall[:, ts:te], d2[:, ts:te], AF.Sqrt)

        def tiles(bi):
            ts, te = bounds[bi]
            for t in range(ts, te):
                ps = mm_pool.tile([P, C], F32, name="ps")
                lhsT = eTa[:, t * P:(t + 1) * P]
                for k in range(NB):
                    nc.tensor.matmul(
                        ps[:, k * FD:(k + 1) * FD],
                        lhsT=lhsT,
                        rhs=chat[:, k * FD:(k + 1) * FD],
                        start=True, stop=True,
                    )
                xz = dist_pool.tile([P, C], BF16, name="dist")
                nc.scalar.activation(
                    xz[:], ps[:], AF.Sqrt,
                    bias=e2[:, t:t + 1], scale=1.0,
                )
                zi = nc.vector.tensor_scalar(
                    out=xz[:], in0=xz[:],
                    scalar1=dall[:, t:t + 1], scalar2=dall[:, t:t + 1],
                    op0=OP.min, op1=OP.subtract,
                )
                z_insts.append(zi)
                if t in A_TILES:
                    nc.scalar.activation(
                        xz[:], xz[:], AF.Identity,
                        accum_out=macc[:, t:t + 1],
                    )
                elif t in POOL_TILES:
                    if state["np"] == 0:
                        nc.gpsimd.tensor_copy(zaccs[4][:], xz[:])
                    else:
                        nc.gpsimd.tensor_add(zaccs[4][:], zaccs[4][:],
                                             xz[:])
                    state["np"] += 1
                else:
                    nh = state["nh"]
                    if nh == NSPLIT:
                        nc.vector.tensor_add(zaccs[0][:], zaccs[0][:],
                                             zaccs[1][:])
                    za = zaccs[nh % 2] if nh < NSPLIT else zaccs[2 + nh % 2]
                    first = nh < 2 or (NSPLIT <= nh < NSPLIT + 2)
                    if first:
                        nc.vector.tensor_copy(za[:], xz[:])
                    else:
                        nc.vector.tensor_add(za[:], za[:], xz[:])
                    state["nh"] += 1

        for bi in range(len(bounds)):
            pre(bi)
            tiles(bi)

        mm_ctx.__exit__(None, None, None)

        # merge accumulators, reduce, negate
        nc.vector.tensor_add(zaccs[2][:], zaccs[2][:], zaccs[3][:])
        nc.vector.tensor_add(zaccs[0][:], zaccs[0][:], zaccs[2][:])
        nc.vector.tensor_reduce(rowtot[:], macc[:], axis=AX.X, op=OP.add)
        nc.vector.tensor_reduce(zrow[:], zaccs[0][:], axis=AX.X, op=OP.add)
        nc.vector.tensor_add(rowtot[:], rowtot[:], zrow[:])
        nc.vector.tensor_scalar_mul(rowtot[:], rowtot[:], -1.0)
        with tc.tile_pool(name="fin", bufs=1, space="PSUM") as finp:
            fin = finp.tile([1, 1], F32)
            nc.tensor.matmul(fin[:], lhsT=rowtot[:], rhs=onesf[:],
                             start=True, stop=True)
            nc.scalar.copy(outsb[:], fin[:])
        nc.sync.dma_start(out, outsb[:])


_NC_CACHE = {}


def build_nc():
    if "nc" in _NC_CACHE:
        return _NC_CACHE["nc"]
    nc = bacc.Bacc(
        "TRN2", target_bir_lowering=False, debug=False, enable_asserts=False
    )
    eT = nc.dram_tensor("eT", [KA, NS], MM_DT, kind="ExternalInput").ap()
    enat = nc.dram_tensor("enat", [P, T * D], BF16, kind="ExternalInput").ap()
    labT = nc.dram_tensor("labT", [P, T], I32, kind="ExternalInput").ap()
    cTb = nc.dram_tensor("cTb", [D, C], BF16, kind="ExternalInput").ap()
    cnat = nc.dram_tensor("cnat", [C, D], F32, kind="ExternalInput").ap()
    out = nc.dram_tensor("out", [1, 1], F32, kind="ExternalOutput").ap()
    with nc.allow_low_precision(reason="bf16 distance pipeline"):
        with tile.TileContext(nc) as tc:
            _body(tc, out, eT, enat, labT, cTb, cnat)
    nc.compile()
    _NC_CACHE["nc"] = nc
    return nc


def make_in_maps(embeddings, centers, labels):
    e = np.ascontiguousarray(np.asarray(embeddings, dtype=np.float32))
    c = np.ascontiguousarray(np.asarray(centers, dtype=np.float32))
    lab = np.asarray(labels).astype(np.int32)
    assert e.shape == (N, D) and c.shape == (C, D) and lab.shape == (N,)
    cTb = np.ascontiguousarray(c.T).astype(ml_dtypes.bfloat16)
    in_maps = []
    for core in range(NCORES):
        es = e[core * NS:(core + 1) * NS]
        ls = lab[core * NS:(core + 1) * NS]
        eT65 = np.ones((KA, NS), np.float32)
        eT65[0:D] = -2.0 * es.T
        eT65 = eT65.astype(ml_dtypes.bfloat16)
        in_maps.append({
            "eT": eT65,
            "enat": np.ascontiguousarray(
                es.reshape(T, P, D).transpose(1, 0, 2).reshape(P, T * D)
                .astype(ml_dtypes.bfloat16)),
            "labT": np.ascontiguousarray(ls.reshape(T, P).T),
            "cTb": cTb,
            "cnat": c,
        })
    return in_maps


def run(embeddings, centers, labels, **kw):
    nc = build_nc()
    in_maps = make_in_maps(embeddings, centers, labels)
    res = run_bass_kernel_spmd(nc, in_maps, core_ids=list(range(NCORES)), **kw)
    total = float(sum(float(r["out"][0, 0]) for r in res.results))
    return np.float32(total), res


def kernel(embeddings, centers, labels):
    val, _ = run(embeddings, centers, labels)
    return val


# revision 15
# speedup vs baseline: 1.2235x; 1.0208x over previous
"""BallLoss Trainium2 kernel (8-core data-parallel SPMD), v5.

loss = sum_{i,j} relu(d_i - d_ij),  d_ij = ||e_i - c_j||, d_i = d_{i,label_i}

Per-core (rows sharded along N across 8 cores, centers replicated):

  - PE:   p[i,j] = c2_j - 2*e_i.c_j via an augmented bf16 matmul, K=65:
          lhsT = [-2*e_i; 1] (host supplies the -2 scale on the e side),
          rhs  = [c^T; c2] (c^T DMA'd straight from the host in bf16;
          c2 computed on-device into psum partition 64 via ones-matmuls
          and copied across to chat row 64 on the same partition).
  - ACT:  dist = sqrt(p + e2_i) (bias = e2 per partition), PSUM -> SBUF
          bf16, one op per [128, 2048] row-tile (~1.86us, the ACT rate
          is 1x from fp32 PSUM).
  - DVE:  z' = (dist min d_i) - d_i = -relu(d_i - dist), one fused
          tensor_scalar at the 4x bf16 rate (~0.73us). The loss is the
          plain GRAND sum of z', so most tiles (H) then do a 2x
          tensor_tensor add into one of several persistent bf16
          accumulators (~1.23us); accumulating the RELU (mean ~-0.56)
          instead of min (~11) keeps bf16 rounding bias ~1e-4 instead
          of a catastrophic 0.5% on the cancelling min-sum. A few
          tiles (A) instead go ACT Identity+accum -> macc, and a few
          H adds run on the otherwise idle GpSimd engine, balancing
          ACT ~ DVE ~ 140us.
  - d_i:  from per-tile indirect-DMA gathers of c[label] (fp32):
          d2_i = sum_d (e_id - c_{lab_i,d})^2 in bf16, d_i = sqrt(d2_i).
  - final: loss = -(sum macc + sum of all accumulators).

Scheduling: input DMAs up front (cTb first: it gates the chat build);
the gather + d2 precompute chain for each 8-tile group is emitted a
full group AHEAD of that group's main tiles so the in-order ACT/DVE
streams never wait on the (mismodeled, slow) indirect gathers; explicit
ordering deps still pin each chain behind earlier main DVE ops so the
scheduler cannot hoist it into the startup. A-tiles are placed at the
very end so the final accumulator merges/reduce overlap their ACT work.

Host: shards inputs, provides layouts/casts only (e^T scaled by -2 in
bf16, ones row, bf16 e, labels as int32), sums the 8 per-core scalars.
"""

from contextlib import ExitStack

import ml_dtypes
import numpy as np

import concourse.bass as bass
import concourse.tile as tile
from concourse import bacc, mybir
from concourse.bass_utils import run_bass_kernel_spmd

F32 = mybir.dt.float32
BF16 = mybir.dt.bfloat16
I32 = mybir.dt.int32
AF = mybir.ActivationFunctionType
OP = mybir.AluOpType
AX = mybir.AxisListType

N, C, D = 65536, 2048, 64
NCORES = 8
NS = N // NCORES  # 8192 rows per core
P = 128           # partitions
T = NS // P       # 64 row-tiles per core
FD = 512          # fp32 psum bank free dim
NB = C // FD      # 4 matmuls per row-tile
G = 8             # row-tiles per precompute group
NG = T // G       # 8 groups

MM_DT = BF16
KA = D + 1        # 64 e dims + c2 ones row

# ACT-heavy tiles (ACT id+accum). The last ones overlap the tail merges.
A_TILES = frozenset({4, 12, 20, 28, 36, 44, 52, 61, 63})
# H tiles whose accumulator add runs on GpSimd (own accumulator)
POOL_TILES = frozenset()
NSPLIT = 24       # DVE H-tiles before switching accumulator pair


def _body(tc, out, eT, enat, labT, cTb, cnat):
    nc = tc.nc
    with ExitStack() as ctx:
        const = ctx.enter_context(tc.tile_pool(name="const", bufs=1))

        eTa = const.tile([KA, NS], MM_DT)    # [65, 8192]
        chat = const.tile([KA, C], MM_DT)    # [65, 2048]: c^T rows + c2
        csqb = const.tile([D, C], BF16)
        ensb = const.tile([P, T * D], BF16)  # e natural, tile-major
        clab = const.tile([P, T * D], F32)   # gathered centers per row
        cdif = const.tile([P, T * D], BF16)  # e - c[label], bf16
        scrb = const.tile([P, T * D], BF16)  # scratch squares
        labsb = const.tile([P, T], I32)
        ones = const.tile([P, 1], BF16)
        onesf = const.tile([P, 1], F32)
        zaccs = [const.tile([P, C], BF16, name=f"zacc{i}") for i in range(4)]
        e2 = const.tile([P, T], F32)
        d2 = const.tile([P, T], F32)
        dall = const.tile([P, T], F32)
        macc = const.tile([P, T], F32)
        rowtot = const.tile([P, 1], F32)
        zrow = const.tile([P, 1], F32)
        outsb = const.tile([1, 1], F32)

        # DMA queue order tuned for the startup critical paths:
        # labsb gates the (slow) indirect gathers; ensb g0 gates the d2
        # chain; cTb gates the chat build; eTa g0 gates the first mains.
        def dma_ensb(g):
            fs, fe = g * G * D, (g + 1) * G * D
            nc.sync.dma_start(ensb[:, fs:fe], enat[:, fs:fe])

        def dma_eTa(g):
            cs, ce = g * G * P, (g + 1) * G * P
            nc.sync.dma_start(eTa[:, cs:ce], eT[:, cs:ce])

        nc.sync.dma_start(labsb[:], labT)
        nc.sync.dma_start(chat[0:D, :], cTb)
        dma_ensb(0)
        dma_eTa(0)
        nc.vector.memset(ones[:], 1.0)
        nc.vector.memset(onesf[:], 1.0)
        nc.vector.memset(macc[:], 0.0)
        for g in range(1, NG):
            dma_ensb(g)
            dma_eTa(g)

        mm_ctx = tc.tile_pool(name="mm", bufs=2, space="PSUM")
        mm_pool = mm_ctx.__enter__()

        # c2 row: csq = (c^T)^2, ones-matmul column sums into psum
        # partition 64, one copy across to chat row 64 (same partition).
        c2ps_full = mm_pool.tile([P, C], F32, name="ps", tag="ps")
        nc.vector.tensor_mul(csqb[:], chat[0:D, :], chat[0:D, :])
        for k in range(NB):
            sl = slice(k * FD, (k + 1) * FD)
            nc.tensor.matmul(
                c2ps_full[64:65, sl], lhsT=ones[0:D, :], rhs=csqb[:, sl],
                start=True, stop=True,
            )
        c2_i = nc.vector.tensor_copy(chat[D:KA, :], c2ps_full[64:65, :])

        dist_pool = ctx.enter_context(tc.tile_pool(name="dist", bufs=6))
        from concourse.tile import add_dep_helper

        z_insts = []
        state = {"nh": 0, "np": 0, "prev_sub": None}
        bounds = [(4 * k, 4 * (k + 1)) for k in range(T // 4)]

        def pre(bi):
            ts, te = bounds[bi]
            fs, fe = ts * D, te * D
            for t in range(ts, te):
                gi_ = nc.gpsimd.indirect_dma_start(
                    out=clab[:, t * D:(t + 1) * D],
                    out_offset=None,
                    in_=cnat,
                    in_offset=bass.IndirectOffsetOnAxis(
                        ap=labsb[:, t:t + 1], axis=0),
                )
                if state.get("prev_sub") is not None:
                    add_dep_helper(gi_.ins, state["prev_sub"].ins, sync=False,
                                   reason="keep pool queue in batch order")
            nc.vector.tensor_mul(scrb[:, fs:fe], ensb[:, fs:fe],
                                 ensb[:, fs:fe])
            nc.vector.tensor_reduce(
                e2[:, ts:te],
                scrb[:, fs:fe].rearrange("p (t d) -> p t d", d=D),
                axis=AX.X, op=OP.add,
            )
            state["prev_sub"] = nc.gpsimd.tensor_sub(
                cdif[:, fs:fe], ensb[:, fs:fe], clab[:, fs:fe]
            )
            sub_i = nc.vector.tensor_mul(
                scrb[:, fs:fe], cdif[:, fs:fe], cdif[:, fs:fe]
            )
            if len(z_insts) >= 3:
                add_dep_helper(sub_i.ins, z_insts[-3].ins, sync=False,
                               reason="hold d2 chain behind prior tiles")
            else:
                add_dep_helper(sub_i.ins, c2_i.ins, sync=False,
                               reason="hold early d2 chain behind chat")
            nc.vector.tensor_reduce(
                d2[:, ts:te],
                scrb[:, fs:fe].rearrange("p (t d) -> p t d", d=D),
                axis=AX.X, op=OP.add,
            )
            nc.scalar.activation(dall[:, ts:te], d2[:, ts:te], AF.Sqrt)

        def tiles(bi):
            ts, te = bounds[bi]
            for t in range(ts, te):
                ps = mm_pool.tile([P, C], F32, name="ps")
                lhsT = eTa[:, t * P:(t + 1) * P]
                for k in range(NB):
                    nc.tensor.matmul(
                        ps[:, k * FD:(k + 1) * FD],
                        lhsT=lhsT,
                        rhs=chat[:, k * FD:(k + 1) * FD],
                        start=True, stop=True,
                    )
                xz = dist_pool.tile([P, C], BF16, name="dist")
                nc.scalar.activation(
                    xz[:], ps[:], AF.Sqrt,
                    bias=e2[:, t:t + 1], scale=1.0,
                )
                zi = nc.vector.tensor_scalar(
                    out=xz[:], in0=xz[:],
                    scalar1=dall[:, t:t + 1], scalar2=dall[:, t:t + 1],
                    op0=OP.min, op1=OP.subtract,
                )
                z_insts.append(zi)
                if t in A_TILES:
                    nc.scalar.activation(
                        xz[:], xz[:], AF.Identity,
                        accum_out=macc[:, t:t + 1],
                    )
                elif t in POOL_TILES:
                    if state["np"] == 0:
                        nc.gpsimd.tensor_copy(zaccs[4][:], xz[:])
                    else:
                        nc.gpsimd.tensor_add(zaccs[4][:], zaccs[4][:],
                                             xz[:])
                    state["np"] += 1
                else:
                    nh = state["nh"]
                    if nh == NSPLIT:
                        nc.vector.tensor_add(zaccs[0][:], zaccs[0][:],
                                             zaccs[1][:])
                    za = zaccs[nh % 2] if nh < NSPLIT else zaccs[2 + nh % 2]
                    first = nh < 2 or (NSPLIT <= nh < NSPLIT + 2)
                    if first:
                        nc.vector.tensor_copy(za[:], xz[:])
                    else:
                        nc.vector.tensor_add(za[:], za[:], xz[:])
                    state["nh"] += 1

        for bi in range(len(bounds)):
            pre(bi)
            tiles(bi)

        mm_ctx.__exit__(None, None, None)

        # merge accumulators, reduce, negate
        nc.vector.tensor_add(zaccs[2][:], zaccs[2][:], zaccs[3][:])
        nc.vector.tensor_add(zaccs[0][:], zaccs[0][:], zaccs[2][:])
        nc.vector.tensor_reduce(rowtot[:], macc[:], axis=AX.X, op=OP.add)
        nc.vector.tensor_reduce(zrow[:], zaccs[0][:], axis=AX.X, op=OP.add)
        nc.vector.tensor_add(rowtot[:], rowtot[:], zrow[:])
        nc.vector.tensor_scalar_mul(rowtot[:], rowtot[:], -1.0)
        with tc.tile_pool(name="fin", bufs=1, space="PSUM") as finp:
            fin = finp.tile([1, 1], F32)
            nc.tensor.matmul(fin[:], lhsT=rowtot[:], rhs=onesf[:],
                             start=True, stop=True)
            nc.scalar.copy(outsb[:], fin[:])
        nc.sync.dma_start(out, outsb[:])


_NC_CACHE = {}


def build_nc():
    if "nc" in _NC_CACHE:
        return _NC_CACHE["nc"]
    nc = bacc.Bacc(
        "TRN2", target_bir_lowering=False, debug=False, enable_asserts=False
    )
    eT = nc.dram_tensor("eT", [KA, NS], MM_DT, kind="ExternalInput").ap()
    enat = nc.dram_tensor("enat", [P, T * D], BF16, kind="ExternalInput").ap()
    labT = nc.dram_tensor("labT", [P, T], I32, kind="ExternalInput").ap()
    cTb = nc.dram_tensor("cTb", [D, C], BF16, kind="ExternalInput").ap()
    cnat = nc.dram_tensor("cnat", [C, D], F32, kind="ExternalInput").ap()
    out = nc.dram_tensor("out", [1, 1], F32, kind="ExternalOutput").ap()
    with nc.allow_low_precision(reason="bf16 distance pipeline"):
        with tile.TileContext(nc) as tc:
            _body(tc, out, eT, enat, labT, cTb, cnat)
    nc.compile()
    _NC_CACHE["nc"] = nc
    return nc


def make_in_maps(embeddings, centers, labels):
    e = np.ascontiguousarray(np.asarray(embeddings, dtype=np.float32))
    c = np.ascontiguousarray(np.asarray(centers, dtype=np.float32))
    lab = np.asarray(labels).astype(np.int32)
    assert e.shape == (N, D) and c.shape == (C, D) and lab.shape == (N,)
    cTb = np.ascontiguousarray(c.T).astype(ml_dtypes.bfloat16)
    in_maps = []
    for core in range(NCORES):
        es = e[core * NS:(core + 1) * NS]
        ls = lab[core * NS:(core + 1) * NS]
        eT65 = np.ones((KA, NS), np.float32)
        eT65[0:D] = -2.0 * es.T
        eT65 = eT65.astype(ml_dtypes.bfloat16)
        in_maps.append({
            "eT": eT65,
            "enat": np.ascontiguousarray(
                es.reshape(T, P, D).transpose(1, 0, 2).reshape(P, T * D)
                .astype(ml_dtypes.bfloat16)),
            "labT": np.ascontiguousarray(ls.reshape(T, P).T),
            "cTb": cTb,
            "cnat": c,
        })
    return in_maps


def run(embeddings, centers, labels, **kw):
    nc = build_nc()
    in_maps = make_in_maps(embeddings, centers, labels)
    res = run_bass_kernel_spmd(nc, in_maps, core_ids=list(range(NCORES)), **kw)
    total = float(sum(float(r["out"][0, 0]) for r in res.results))
    return np.float32(total), res


def kernel(embeddings, centers, labels):
    val, _ = run(embeddings, centers, labels)
    return val
